# revision 8
# baseline (speedup 1.0000x reference)
"""Gated attention-with-pair-bias kernel for 8 Trainium2 NeuronCores.

Problem: B=2, Q=K=2048, C=256, H=8 heads, D=32 per head.
  q = (q_x @ Wq.T)/sqrt(D); k = kv_x @ Wk.T; v = kv_x @ Wv.T   (per head h)
  S = q @ k.T + bias_mask + bias_pair; w = softmax_k(S)
  o = (w @ v) * sigmoid(q_x @ Wg.T + bg); out = o @ Wo.T + bo

Sharding: one head per core (8 heads / 8 cores); each core handles both
batch elements, so each head's 16.8MB bias_pair slice streams from HBM
exactly once.  The per-head output projection partials are summed on host.

On-chip layout is "ST" (scores transposed): S.T tiles are [k->128
partitions, q->512 free].  Benefits:
  - softmax denominator l[q] falls out of the o-matmul as a ones-column
    appended to v (contract over k = partitions),
  - w feeds the o-matmul directly as the moving operand (no transposes),
  - bias_mask[b,k] is a per-partition scalar -> folded into the ACT Exp
    instruction's bias input for free.
bias_pair arrives host-transposed as bpT [K, Q].  All matmuls run in
float32r (fp32 storage, ~1e-4 rel precision, 1 cycle/row at N>=512).
The D=32 contraction of the score matmuls is packed 4x with PE row
tiling (tile_position); q.T is replicated across the 4 partition strips
by 4x col-tiled projection matmuls.
"""

import math
import sys

sys.path.insert(0, "/opt/trn_rl_repo")

import numpy as np

H, D, B, Q, K, C = 8, 32, 2, 2048, 2048, 256
NQC = 4          # q chunks of 512
NKT = K // 128   # 16 k tiles

_CACHE = {}


def _build():
    import concourse.bacc as bacc
    import concourse.mybir as mybir
    from concourse.tile import TileContext

    F32 = mybir.dt.float32
    F32R = mybir.dt.float32r
    BF16 = mybir.dt.bfloat16
    EXP = mybir.ActivationFunctionType.Exp
    SIG = mybir.ActivationFunctionType.Sigmoid
    ADD = mybir.AluOpType.add
    MULT = mybir.AluOpType.mult

    nc = bacc.Bacc(None, target_bir_lowering=False)
    qxT = nc.dram_tensor("qxT", [B, 2, 128, Q], F32R, kind="ExternalInput")
    kvT = nc.dram_tensor("kvT", [B, 2, 128, K], F32R, kind="ExternalInput")
    bpT = nc.dram_tensor("bpT", [K, Q], BF16, kind="ExternalInput")
    ident = nc.dram_tensor("ident", [128, 128], BF16, kind="ExternalInput")
    expbm = nc.dram_tensor("expbm", [B, 128, NKT], F32, kind="ExternalInput")
    wq = nc.dram_tensor("wq", [2, 128, 128], F32R, kind="ExternalInput")
    wk = nc.dram_tensor("wk", [4, 2, 128, 128], F32R, kind="ExternalInput")
    wv = nc.dram_tensor("wv", [2, 128, D], F32R, kind="ExternalInput")
    wg = nc.dram_tensor("wg", [2, 128, D], F32R, kind="ExternalInput")
    wo = nc.dram_tensor("wo", [D, C], F32R, kind="ExternalInput")
    bgv = nc.dram_tensor("bgv", [D, 1], F32, kind="ExternalInput")
    ones1 = nc.dram_tensor("ones1", [33, 32], F32R, kind="ExternalInput")
    onesv = nc.dram_tensor("onesv", [128, NKT], BF16, kind="ExternalInput")
    outT = nc.dram_tensor("outT", [B, 2, 128, Q], F32, kind="ExternalOutput")

    with TileContext(nc) as tc:
        with (
            tc.tile_pool(name="ld", bufs=1) as ld,
            tc.tile_pool(name="pers", bufs=1) as pers,
            tc.tile_pool(name="bp", bufs=12) as bppool,
            tc.tile_pool(name="wp", bufs=6) as wpool,
            tc.tile_pool(name="ob", bufs=2) as obpool,
            tc.tile_pool(name="ps_sc", bufs=2, space="PSUM") as ps_sc,
            tc.tile_pool(name="ps_o", bufs=2, space="PSUM") as ps_o,
            tc.tile_pool(name="ps_m", bufs=2, space="PSUM") as ps_m,
        ):
            # ---- constants & weights ----
            ones_sb = pers.tile([33, 32], F32R, name="ones_sb")
            nc.sync.dma_start(out=ones_sb[:, :], in_=ones1[:, :])
            bg_sb = pers.tile([D, 1], F32, name="bg_sb")
            nc.sync.dma_start(out=bg_sb[:, :], in_=bgv[:, :])
            id_sb = pers.tile([128, 128], BF16, name="id_sb")
            nc.sync.dma_start(out=id_sb[:, :], in_=ident[:, :])
            wo_sb = pers.tile([D, C], F32R, name="wo_sb")
            nc.sync.dma_start(out=wo_sb[:, :], in_=wo[:, :])
            bm_sb, wq_sb, wk_sb, wv_sb, wg_sb = [], [], [], [], []
            for b in range(B):
                t = pers.tile([128, NKT], F32, name=f"bm_sb{b}")
                nc.sync.dma_start(out=t[:, :], in_=expbm[b, :, :])
                bm_sb.append(t)
            for ch in range(2):
                t = pers.tile([128, 128], F32R, name=f"wq_sb{ch}")
                nc.sync.dma_start(out=t[:, :], in_=wq[ch, :, :])
                wq_sb.append(t)
                for lst, src, nm in ((wv_sb, wv, "wv"), (wg_sb, wg, "wg")):
                    t = pers.tile([128, D], F32R, name=f"{nm}_sb{ch}")
                    nc.sync.dma_start(out=t[:, :], in_=src[ch, :, :])
                    lst.append(t)
            for j in range(4):
                row = []
                for ch in range(2):
                    t = pers.tile([128, 128], F32R, name=f"wk_sb{j}{ch}")
                    nc.sync.dma_start(out=t[:, :], in_=wk[j, ch, :, :])
                    row.append(t)
                wk_sb.append(row)

            # ---- per-batch projections ----
            qT_rep, kT_sb, v_sb, gT_sb = [], [], [], []
            for b in range(B):
                qx_b, kv_b = [], []
                for ch in range(2):
                    t = ld.tile([128, Q], F32R, name=f"qx{b}{ch}", tag=f"qx{ch}", bufs=2)
                    nc.sync.dma_start(out=t[:, :], in_=qxT[b, ch, :, :])
                    qx_b.append(t)
                    t = ld.tile([128, K], F32R, name=f"kv{b}{ch}", tag=f"kv{ch}", bufs=2)
                    nc.sync.dma_start(out=t[:, :], in_=kvT[b, ch, :, :])
                    kv_b.append(t)
                qT = pers.tile([128, Q], BF16, name=f"qT{b}")
                kT = pers.tile([128, 512], BF16, name=f"kT{b}")
                vt = pers.tile([128, NKT, 33], BF16, name=f"v{b}")
                gT = pers.tile([32, Q], F32, name=f"gT{b}")
                qT_rep.append(qT); kT_sb.append(kT); v_sb.append(vt); gT_sb.append(gT)

                # q.T replicated into 4 partition strips via 4x-duplicated
                # weight columns (host-prepared) -> plain M=128 matmuls
                for qc in range(NQC):
                    ps = ps_sc.tile([128, 512], F32, tag="sc", name=f"pq{b}{qc}")
                    for ch in range(2):
                        nc.tensor.matmul(
                            ps[:, :], wq_sb[ch][:, :],
                            qx_b[ch][:, qc * 512:(qc + 1) * 512],
                            start=(ch == 0), stop=(ch == 1))
                    nc.vector.tensor_copy(qT[:, qc * 512:(qc + 1) * 512], ps[:, :])

                # k.T in strip layout (strip j holds k tiles {4g+j : g}) via
                # zero-padded strip weights accumulating into one bank
                ps = ps_sc.tile([128, 512], F32, tag="sc", name=f"pk{b}")
                for j in range(4):
                    kv_r = [t.rearrange("p (g j i) -> p j g i", g=4, i=128)
                            for t in kv_b]
                    for ch in range(2):
                        nc.tensor.matmul(
                            ps[:, :], wk_sb[j][ch][:, :], kv_r[ch][:, j],
                            start=(j == 0 and ch == 0), stop=(j == 3 and ch == 1))
                nc.vector.tensor_copy(kT[:, :], ps[:, :])

                # v in [k-partitions, d] layout + ones column for the softmax sum
                for kt in range(NKT):
                    ps = ps_m.tile([128, 32], F32, tag="m", name=f"pv{b}{kt}",
                                   padded_shape=[128, 512])
                    for ch in range(2):
                        nc.tensor.matmul(
                            ps[:, :], kv_b[ch][:, kt * 128:(kt + 1) * 128],
                            wv_sb[ch][:, :], start=(ch == 0), stop=(ch == 1))
                    nc.vector.tensor_copy(vt[:, kt, 0:32], ps[:, :])
                nc.sync.dma_start(out=vt[:, :, 32], in_=onesv[:, :])
                # fold bias_mask into v: softmax numerator and denominator
                # both scale by exp(bm[k]) per k-partition row
                for kt in range(NKT):
                    nc.vector.tensor_scalar_mul(vt[:, kt, :], vt[:, kt, :],
                                                bm_sb[b][:, kt:kt + 1])

                # gate.T = sigmoid(Wg_h @ qx.T + bg)
                for qc in range(NQC):
                    ps = ps_m.tile([32, 512], F32, tag="m", name=f"pg{b}{qc}",
                                   padded_shape=[128, 512])
                    for ch in range(2):
                        nc.tensor.matmul(
                            ps[:, :], wg_sb[ch][:, :],
                            qx_b[ch][:, qc * 512:(qc + 1) * 512],
                            start=(ch == 0), stop=(ch == 1))
                    nc.scalar.activation(gT[:, qc * 512:(qc + 1) * 512], ps[:, :],
                                         SIG, bias=bg_sb[:, :])

            # ---- main attention loop ----
            pending = None
            # bpT DRAM viewed as [kt, p, q] for paired-tile loads
            bp_r = bpT.rearrange("(kt p) q -> kt p q", p=128)
            for qc in range(NQC):
                bts = []
                for kp in range(NKT // 2):
                    # [128, 2, 512]: two k-tiles' bias for this q chunk
                    t = bppool.tile([128, 2, 512], BF16, tag="bp", name=f"bp{qc}_{kp}")
                    nc.sync.dma_start(
                        out=t[:, :, :],
                        in_=bp_r[2 * kp:2 * kp + 2, :,
                                 qc * 512:(qc + 1) * 512].rearrange(
                                     "kt p q -> p kt q"))
                    bts.append(t)
                for b in range(B):
                    po = ps_o.tile([33, 512], F32, tag="o", name=f"po{qc}{b}")
                    for g in range(4):
                        psg = []
                        for half in range(2):
                            # one 2-bank tile holds strips {2*half, 2*half+1}
                            ps = ps_sc.tile([128, 2, 512], F32, tag="sc",
                                            name=f"s{qc}{b}{g}{half}")
                            for jj in range(2):
                                j = 2 * half + jj
                                # bank <- I @ bias_pair tile, then scores
                                # accumulate on top (bias add on the PE)
                                nc.tensor.matmul(
                                    ps[:, jj, :], id_sb[:, :],
                                    bts[2 * g + half][:, jj, :],
                                    start=True, stop=False)
                                nc.tensor.matmul(
                                    ps[:, jj, :],
                                    kT_sb[b][32 * j:32 * j + 32, g * 128:(g + 1) * 128],
                                    qT_rep[b][32 * j:32 * j + 32, qc * 512:(qc + 1) * 512],
                                    start=False, stop=True, tile_position=(32 * j, 0))
                            psg.append(ps)
                        if g == 0 and pending is not None:
                            pending(); pending = None
                        for half in range(2):
                            wt = wpool.tile([128, 2, 512], BF16, tag="w",
                                            name=f"w{qc}{b}{g}{half}")
                            nc.scalar.activation(wt[:, :, :], psg[half][:, :, :], EXP)
                            for jj in range(2):
                                kt = 4 * g + 2 * half + jj
                                nc.tensor.matmul(po[:, :], v_sb[b][:, kt, :],
                                                 wt[:, jj, :],
                                                 start=(kt == 0), stop=(kt == NKT - 1))

                    def make_epilogue(qc, b, po):
                        def epilogue():
                            # evacuate po once, broadcast l, normalize+gate+project
                            posb = wpool.tile([33, 512], F32R, tag="posb",
                                              name=f"posb{qc}{b}", bufs=2)
                            nc.vector.tensor_copy(posb[:, :], po[:, :])
                            pbc = ps_m.tile([32, 512], F32, tag="m", name=f"bc{qc}{b}",
                                            padded_shape=[128, 512])
                            nc.tensor.matmul(pbc[:, :], ones_sb[32:33, :],
                                             posb[32:33, :],
                                             start=True, stop=True, tile_position=(32, 0))
                            rl = wpool.tile([32, 512], F32, tag="rl",
                                            name=f"rl{qc}{b}", bufs=2)
                            nc.vector.reciprocal_approx_fast(rl[:, :], pbc[:, :])
                            og = wpool.tile([32, 512], F32R, tag="og",
                                            name=f"og{qc}{b}", bufs=2)
                            nc.vector.tensor_tensor(
                                og[:, :], gT_sb[b][:, qc * 512:(qc + 1) * 512],
                                posb[0:32, :], op=MULT)
                            nc.vector.tensor_tensor(og[:, :], og[:, :], rl[:, :],
                                                    op=MULT)
                            for half in range(2):
                                pp = ps_m.tile([128, 512], F32, tag="m",
                                               name=f"pp{qc}{b}{half}")
                                nc.tensor.matmul(
                                    pp[:, :], wo_sb[:, half * 128:(half + 1) * 128],
                                    og[:, :], start=True, stop=True)
                                ot = obpool.tile([128, 512], F32, tag="ot",
                                                 name=f"ot{qc}{b}{half}")
                                nc.scalar.copy(ot[:, :], pp[:, :])
                                nc.sync.dma_start(
                                    out=outT[b, half, :, qc * 512:(qc + 1) * 512],
                                    in_=ot[:, :])
                        return epilogue

                    pending = make_epilogue(qc, b, po)
            pending(); pending = None
    nc.compile()
    return nc


def _get_nc():
    if "nc" not in _CACHE:
        _CACHE["nc"] = _build()
    return _CACHE["nc"]


def kernel(q_x, kv_x, bias_mask, bias_pair, Wq, Wk, Wv, Wo, bo, Wg, bg):
    from concourse.bass_utils import run_bass_kernel_spmd

    nc = _get_nc()
    f32 = np.float32
    q_x = np.asarray(q_x, f32); kv_x = np.asarray(kv_x, f32)
    bias_mask = np.asarray(bias_mask, f32); bias_pair = np.asarray(bias_pair, f32)
    Wq = np.asarray(Wq, f32); Wk = np.asarray(Wk, f32); Wv = np.asarray(Wv, f32)
    Wo = np.asarray(Wo, f32); bo = np.asarray(bo, f32); Wg = np.asarray(Wg, f32)
    bg = np.asarray(bg, f32)

    import ml_dtypes
    _bf16 = ml_dtypes.bfloat16
    sD = 1.0 / math.sqrt(D)
    qxT_dev = np.ascontiguousarray(
        q_x.transpose(0, 2, 1).reshape(B, 2, 128, Q))
    kvT_dev = np.ascontiguousarray(
        kv_x.transpose(0, 2, 1).reshape(B, 2, 128, K))
    bm_dev = np.ascontiguousarray(
        np.exp(bias_mask.reshape(B, NKT, 128).transpose(0, 2, 1)))

    def wsplit(W, h, scale=1.0):
        # [2, 128, D] view of (W_h * scale).T with W_h = W[h*D:(h+1)*D, :]
        return np.ascontiguousarray(
            (W[h * D:(h + 1) * D, :] * scale).T.reshape(2, 128, D))

    def wrep(W, h, scale=1.0):
        # weight columns duplicated 4x -> M=128 matmul emits 4 replicas
        wt = wsplit(W, h, scale)                       # [2, 128, D]
        return np.ascontiguousarray(np.tile(wt, (1, 1, 4)))

    def wstrips(W, h):
        # strip j: W_h.T placed at columns 32j..32j+32, zeros elsewhere
        wt = wsplit(W, h)                              # [2, 128, D]
        out = np.zeros((4, 2, 128, 128), np.float32)
        for j in range(4):
            out[j, :, :, 32 * j:32 * j + 32] = wt
        return out

    in_maps = []
    for h in range(H):
        in_maps.append({
            "qxT": qxT_dev, "kvT": kvT_dev,
            "bpT": np.ascontiguousarray(bias_pair[0, h].T).astype(_bf16),
            "ident": np.eye(128, dtype=_bf16),
            "expbm": bm_dev,
            "wq": wrep(Wq, h, sD), "wk": wstrips(Wk, h),
            "wv": wsplit(Wv, h), "wg": wsplit(Wg, h),
            "ones1": np.ones((33, 32), np.float32),
            "onesv": np.ones((128, NKT), _bf16),
            "wo": np.ascontiguousarray(Wo[:, h * D:(h + 1) * D].T),
            "bgv": np.ascontiguousarray(bg[h * D:(h + 1) * D, None]),
        })

    res = run_bass_kernel_spmd(nc, in_maps, core_ids=list(range(H)))
    out = np.zeros((B, Q, C), f32)
    for h in range(H):
        p = res.results[h]["outT"].reshape(B, C, Q)
        out += p.transpose(0, 2, 1)
    out += bo
    return out


# revision 9
# speedup vs baseline: 1.5387x; 1.5387x over previous
"""Gated attention-with-pair-bias kernel for 8 Trainium2 NeuronCores.

Problem: B=2, Q=K=2048, C=256, H=8 heads, D=32 per head.
  q = (q_x @ Wq.T)/sqrt(D); k = kv_x @ Wk.T; v = kv_x @ Wv.T   (per head h)
  S = q @ k.T + bias_mask + bias_pair; w = softmax_k(S)
  o = (w @ v) * sigmoid(q_x @ Wg.T + bg); out = o @ Wo.T + bo

Sharding: one head per core (8 heads / 8 cores); each core handles both
batch elements, so each head's 16.8MB bias_pair slice streams from HBM
exactly once.  The per-head output projection partials are summed on host.

On-chip layout is "ST" (scores transposed): S.T tiles are [k->128
partitions, q->512 free].  Benefits:
  - softmax denominator l[q] falls out of the o-matmul as a ones-column
    appended to v (contract over k = partitions),
  - w feeds the o-matmul directly as the moving operand (no transposes),
  - bias_mask[b,k] is a per-partition scalar -> folded into the ACT Exp
    instruction's bias input for free.
bias_pair arrives host-transposed as bpT [K, Q].  All matmuls run in
float32r (fp32 storage, ~1e-4 rel precision, 1 cycle/row at N>=512).
The D=32 contraction of the score matmuls is packed 4x with PE row
tiling (tile_position); q.T is replicated across the 4 partition strips
by 4x col-tiled projection matmuls.
"""

import math
import sys

sys.path.insert(0, "/opt/trn_rl_repo")

import numpy as np

H, D, B, Q, K, C = 8, 32, 2, 2048, 2048, 256
NQC = 4          # q chunks of 512
NKT = K // 128   # 16 k tiles

_CACHE = {}


def _build():
    import concourse.bacc as bacc
    import concourse.mybir as mybir
    from concourse.tile import TileContext

    F32 = mybir.dt.float32
    F32R = mybir.dt.float32r
    BF16 = mybir.dt.bfloat16
    EXP = mybir.ActivationFunctionType.Exp
    SIG = mybir.ActivationFunctionType.Sigmoid
    ADD = mybir.AluOpType.add
    MULT = mybir.AluOpType.mult

    nc = bacc.Bacc(None, target_bir_lowering=False)
    qxT = nc.dram_tensor("qxT", [B, 2, 128, Q], F32R, kind="ExternalInput")
    kvT = nc.dram_tensor("kvT", [B, 2, 128, K], F32R, kind="ExternalInput")
    bpT = nc.dram_tensor("bpT", [K, Q], BF16, kind="ExternalInput")
    ident = nc.dram_tensor("ident", [128, 128], BF16, kind="ExternalInput")
    expbm = nc.dram_tensor("expbm", [B, 128, NKT], F32, kind="ExternalInput")
    wq = nc.dram_tensor("wq", [2, 128, 128], F32R, kind="ExternalInput")
    wk = nc.dram_tensor("wk", [4, 2, 128, 128], F32R, kind="ExternalInput")
    wv = nc.dram_tensor("wv", [2, 128, D], F32R, kind="ExternalInput")
    wg = nc.dram_tensor("wg", [2, 128, D], F32R, kind="ExternalInput")
    wo = nc.dram_tensor("wo", [D, C], F32R, kind="ExternalInput")
    bgv = nc.dram_tensor("bgv", [D, 1], F32, kind="ExternalInput")
    ones1 = nc.dram_tensor("ones1", [33, 32], F32R, kind="ExternalInput")
    onesv = nc.dram_tensor("onesv", [128, NKT], BF16, kind="ExternalInput")
    outT = nc.dram_tensor("outT", [B, 2, 128, Q], F32, kind="ExternalOutput")

    with TileContext(nc) as tc:
        with (
            tc.tile_pool(name="ld", bufs=1) as ld,
            tc.tile_pool(name="pers", bufs=1) as pers,
            tc.tile_pool(name="bp", bufs=12) as bppool,
            tc.tile_pool(name="wp", bufs=6) as wpool,
            tc.tile_pool(name="ob", bufs=2) as obpool,
            tc.tile_pool(name="ps_sc", bufs=2, space="PSUM") as ps_sc,
            tc.tile_pool(name="ps_o", bufs=2, space="PSUM") as ps_o,
            tc.tile_pool(name="ps_m", bufs=2, space="PSUM") as ps_m,
        ):
            # ---- constants & weights ----
            ones_sb = pers.tile([33, 32], F32R, name="ones_sb")
            nc.sync.dma_start(out=ones_sb[:, :], in_=ones1[:, :])
            bg_sb = pers.tile([D, 1], F32, name="bg_sb")
            nc.sync.dma_start(out=bg_sb[:, :], in_=bgv[:, :])
            id_sb = pers.tile([128, 128], BF16, name="id_sb")
            nc.sync.dma_start(out=id_sb[:, :], in_=ident[:, :])
            wo_sb = pers.tile([D, C], F32R, name="wo_sb")
            nc.sync.dma_start(out=wo_sb[:, :], in_=wo[:, :])
            bm_sb, wq_sb, wk_sb, wv_sb, wg_sb = [], [], [], [], []
            for b in range(B):
                t = pers.tile([128, NKT], F32, name=f"bm_sb{b}")
                nc.sync.dma_start(out=t[:, :], in_=expbm[b, :, :])
                bm_sb.append(t)
            for ch in range(2):
                t = pers.tile([128, 128], F32R, name=f"wq_sb{ch}")
                nc.sync.dma_start(out=t[:, :], in_=wq[ch, :, :])
                wq_sb.append(t)
                for lst, src, nm in ((wv_sb, wv, "wv"), (wg_sb, wg, "wg")):
                    t = pers.tile([128, D], F32R, name=f"{nm}_sb{ch}")
                    nc.sync.dma_start(out=t[:, :], in_=src[ch, :, :])
                    lst.append(t)
            for j in range(4):
                row = []
                for ch in range(2):
                    t = pers.tile([128, 128], F32R, name=f"wk_sb{j}{ch}")
                    nc.sync.dma_start(out=t[:, :], in_=wk[j, ch, :, :])
                    row.append(t)
                wk_sb.append(row)

            # ---- per-batch projections ----
            qT_rep, kT_sb, v_sb, gT_sb = [], [], [], []
            for b in range(B):
                qx_b, kv_b = [], []
                for ch in range(2):
                    t = ld.tile([128, Q], F32R, name=f"qx{b}{ch}", tag=f"qx{ch}", bufs=2)
                    nc.sync.dma_start(out=t[:, :], in_=qxT[b, ch, :, :])
                    qx_b.append(t)
                    t = ld.tile([128, K], F32R, name=f"kv{b}{ch}", tag=f"kv{ch}", bufs=2)
                    nc.sync.dma_start(out=t[:, :], in_=kvT[b, ch, :, :])
                    kv_b.append(t)
                qT = pers.tile([128, Q], BF16, name=f"qT{b}")
                kT = pers.tile([128, 512], BF16, name=f"kT{b}")
                vt = pers.tile([128, NKT, 33], BF16, name=f"v{b}")
                gT = pers.tile([32, Q], F32, name=f"gT{b}")
                qT_rep.append(qT); kT_sb.append(kT); v_sb.append(vt); gT_sb.append(gT)

                # q.T replicated into 4 partition strips via 4x-duplicated
                # weight columns (host-prepared) -> plain M=128 matmuls
                for qc in range(NQC):
                    ps = ps_sc.tile([128, 512], F32, tag="sc", name=f"pq{b}{qc}")
                    for ch in range(2):
                        nc.tensor.matmul(
                            ps[:, :], wq_sb[ch][:, :],
                            qx_b[ch][:, qc * 512:(qc + 1) * 512],
                            start=(ch == 0), stop=(ch == 1))
                    nc.vector.tensor_copy(qT[:, qc * 512:(qc + 1) * 512], ps[:, :])

                # k.T in strip layout (strip j holds k tiles {4g+j : g}) via
                # zero-padded strip weights accumulating into one bank
                ps = ps_sc.tile([128, 512], F32, tag="sc", name=f"pk{b}")
                for j in range(4):
                    kv_r = [t.rearrange("p (g j i) -> p j g i", g=4, i=128)
                            for t in kv_b]
                    for ch in range(2):
                        nc.tensor.matmul(
                            ps[:, :], wk_sb[j][ch][:, :], kv_r[ch][:, j],
                            start=(j == 0 and ch == 0), stop=(j == 3 and ch == 1))
                nc.vector.tensor_copy(kT[:, :], ps[:, :])

                # v in [k-partitions, d] layout + ones column for the softmax sum
                for kt in range(NKT):
                    ps = ps_m.tile([128, 32], F32, tag="m", name=f"pv{b}{kt}",
                                   padded_shape=[128, 512])
                    for ch in range(2):
                        nc.tensor.matmul(
                            ps[:, :], kv_b[ch][:, kt * 128:(kt + 1) * 128],
                            wv_sb[ch][:, :], start=(ch == 0), stop=(ch == 1))
                    nc.vector.tensor_copy(vt[:, kt, 0:32], ps[:, :])
                nc.sync.dma_start(out=vt[:, :, 32], in_=onesv[:, :])
                # fold bias_mask into v: softmax numerator and denominator
                # both scale by exp(bm[k]) per k-partition row
                for kt in range(NKT):
                    nc.vector.tensor_scalar_mul(vt[:, kt, :], vt[:, kt, :],
                                                bm_sb[b][:, kt:kt + 1])

                # gate.T = sigmoid(Wg_h @ qx.T + bg)
                for qc in range(NQC):
                    ps = ps_m.tile([32, 512], F32, tag="m", name=f"pg{b}{qc}",
                                   padded_shape=[128, 512])
                    for ch in range(2):
                        nc.tensor.matmul(
                            ps[:, :], wg_sb[ch][:, :],
                            qx_b[ch][:, qc * 512:(qc + 1) * 512],
                            start=(ch == 0), stop=(ch == 1))
                    nc.scalar.activation(gT[:, qc * 512:(qc + 1) * 512], ps[:, :],
                                         SIG, bias=bg_sb[:, :])

            # ---- main attention loop ----
            pending = None
            pending_o = None
            # bpT DRAM viewed as [kt, p, q] for paired-tile loads
            bp_r = bpT.rearrange("(kt p) q -> kt p q", p=128)
            for qc in range(NQC):
                bts = []
                for kp in range(NKT // 2):
                    # [128, 2, 512]: two k-tiles' bias for this q chunk
                    t = bppool.tile([128, 2, 512], BF16, tag="bp", name=f"bp{qc}_{kp}")
                    nc.sync.dma_start(
                        out=t[:, :, :],
                        in_=bp_r[2 * kp:2 * kp + 2, :,
                                 qc * 512:(qc + 1) * 512].rearrange(
                                     "kt p q -> p kt q"))
                    bts.append(t)
                for b in range(B):
                    po = ps_o.tile([33, 512], F32, tag="o", name=f"po{qc}{b}")
                    for g in range(4):
                        psg = []
                        # all four identity-MMs (bias loads) first, then all
                        # four score MMs: batches PE tiling modes (128-row vs
                        # 32-row) to avoid per-MM mode-switch drains
                        for half in range(2):
                            ps = ps_sc.tile([128, 2, 512], F32, tag="sc",
                                            name=f"s{qc}{b}{g}{half}")
                            for jj in range(2):
                                nc.tensor.matmul(
                                    ps[:, jj, :], id_sb[:, :],
                                    bts[2 * g + half][:, jj, :],
                                    start=True, stop=False)
                            psg.append(ps)
                        for half in range(2):
                            for jj in range(2):
                                j = 2 * half + jj
                                nc.tensor.matmul(
                                    psg[half][:, jj, :],
                                    kT_sb[b][32 * j:32 * j + 32, g * 128:(g + 1) * 128],
                                    qT_rep[b][32 * j:32 * j + 32, qc * 512:(qc + 1) * 512],
                                    start=False, stop=True, tile_position=(32 * j, 0))
                        wts = []
                        for half in range(2):
                            wt = wpool.tile([128, 2, 512], BF16, tag="w",
                                            name=f"w{qc}{b}{g}{half}")
                            nc.scalar.activation(wt[:, :, :], psg[half][:, :, :], EXP)
                            wts.append(wt)
                        # o-MMs run one group behind their exp so the PE's
                        # in-order queue never waits on the Scalar engine
                        if pending_o is not None:
                            pending_o(); pending_o = None
                        if g == 0 and pending is not None:
                            pending(); pending = None

                        def make_o(b, g, po, wts):
                            def emit_o():
                                for half in range(2):
                                    for jj in range(2):
                                        kt = 4 * g + 2 * half + jj
                                        nc.tensor.matmul(
                                            po[:, :], v_sb[b][:, kt, :],
                                            wts[half][:, jj, :],
                                            start=(kt == 0), stop=(kt == NKT - 1))
                            return emit_o
                        pending_o = make_o(b, g, po, wts)

                    def make_epilogue(qc, b, po):
                        def epilogue():
                            # evacuate po once, broadcast l, normalize+gate+project
                            posb = wpool.tile([33, 512], F32R, tag="posb",
                                              name=f"posb{qc}{b}", bufs=2)
                            nc.vector.tensor_copy(posb[:, :], po[:, :])
                            pbc = ps_m.tile([32, 512], F32, tag="m", name=f"bc{qc}{b}",
                                            padded_shape=[128, 512])
                            nc.tensor.matmul(pbc[:, :], ones_sb[32:33, :],
                                             posb[32:33, :],
                                             start=True, stop=True, tile_position=(32, 0))
                            rl = wpool.tile([32, 512], F32, tag="rl",
                                            name=f"rl{qc}{b}", bufs=2)
                            nc.vector.reciprocal_approx_fast(rl[:, :], pbc[:, :])
                            og = wpool.tile([32, 512], F32R, tag="og",
                                            name=f"og{qc}{b}", bufs=2)
                            nc.vector.tensor_tensor(
                                og[:, :], gT_sb[b][:, qc * 512:(qc + 1) * 512],
                                posb[0:32, :], op=MULT)
                            nc.vector.tensor_tensor(og[:, :], og[:, :], rl[:, :],
                                                    op=MULT)
                            for half in range(2):
                                pp = ps_m.tile([128, 512], F32, tag="m",
                                               name=f"pp{qc}{b}{half}")
                                nc.tensor.matmul(
                                    pp[:, :], wo_sb[:, half * 128:(half + 1) * 128],
                                    og[:, :], start=True, stop=True)
                                ot = obpool.tile([128, 512], F32, tag="ot",
                                                 name=f"ot{qc}{b}{half}")
                                nc.scalar.copy(ot[:, :], pp[:, :])
                                nc.sync.dma_start(
                                    out=outT[b, half, :, qc * 512:(qc + 1) * 512],
                                    in_=ot[:, :])
                        return epilogue

                    pending = make_epilogue(qc, b, po)
            pending_o(); pending_o = None
            pending(); pending = None
    nc.compile()
    return nc


def _get_nc():
    if "nc" not in _CACHE:
        _CACHE["nc"] = _build()
    return _CACHE["nc"]


def kernel(q_x, kv_x, bias_mask, bias_pair, Wq, Wk, Wv, Wo, bo, Wg, bg):
    from concourse.bass_utils import run_bass_kernel_spmd

    nc = _get_nc()
    f32 = np.float32
    q_x = np.asarray(q_x, f32); kv_x = np.asarray(kv_x, f32)
    bias_mask = np.asarray(bias_mask, f32); bias_pair = np.asarray(bias_pair, f32)
    Wq = np.asarray(Wq, f32); Wk = np.asarray(Wk, f32); Wv = np.asarray(Wv, f32)
    Wo = np.asarray(Wo, f32); bo = np.asarray(bo, f32); Wg = np.asarray(Wg, f32)
    bg = np.asarray(bg, f32)

    import ml_dtypes
    _bf16 = ml_dtypes.bfloat16
    sD = 1.0 / math.sqrt(D)
    qxT_dev = np.ascontiguousarray(
        q_x.transpose(0, 2, 1).reshape(B, 2, 128, Q))
    kvT_dev = np.ascontiguousarray(
        kv_x.transpose(0, 2, 1).reshape(B, 2, 128, K))
    bm_dev = np.ascontiguousarray(
        np.exp(bias_mask.reshape(B, NKT, 128).transpose(0, 2, 1)))

    def wsplit(W, h, scale=1.0):
        # [2, 128, D] view of (W_h * scale).T with W_h = W[h*D:(h+1)*D, :]
        return np.ascontiguousarray(
            (W[h * D:(h + 1) * D, :] * scale).T.reshape(2, 128, D))

    def wrep(W, h, scale=1.0):
        # weight columns duplicated 4x -> M=128 matmul emits 4 replicas
        wt = wsplit(W, h, scale)                       # [2, 128, D]
        return np.ascontiguousarray(np.tile(wt, (1, 1, 4)))

    def wstrips(W, h):
        # strip j: W_h.T placed at columns 32j..32j+32, zeros elsewhere
        wt = wsplit(W, h)                              # [2, 128, D]
        out = np.zeros((4, 2, 128, 128), np.float32)
        for j in range(4):
            out[j, :, :, 32 * j:32 * j + 32] = wt
        return out

    in_maps = []
    for h in range(H):
        in_maps.append({
            "qxT": qxT_dev, "kvT": kvT_dev,
            "bpT": np.ascontiguousarray(bias_pair[0, h].T).astype(_bf16),
            "ident": np.eye(128, dtype=_bf16),
            "expbm": bm_dev,
            "wq": wrep(Wq, h, sD), "wk": wstrips(Wk, h),
            "wv": wsplit(Wv, h), "wg": wsplit(Wg, h),
            "ones1": np.ones((33, 32), np.float32),
            "onesv": np.ones((128, NKT), _bf16),
            "wo": np.ascontiguousarray(Wo[:, h * D:(h + 1) * D].T),
            "bgv": np.ascontiguousarray(bg[h * D:(h + 1) * D, None]),
        })

    res = run_bass_kernel_spmd(nc, in_maps, core_ids=list(range(H)))
    out = np.zeros((B, Q, C), f32)
    for h in range(H):
        p = res.results[h]["outT"].reshape(B, C, Q)
        out += p.transpose(0, 2, 1)
    out += bo
    return out
